# revision 7
# baseline (speedup 1.0000x reference)
"""Trainium2 Bass kernel for nn_DynamicConvolution.

Reference computation (per batch b, T=4096 timesteps, C=512 channels):
    h  = x @ w_in.T + b_in                    # (T, 2C)
    xg = h[:, :C] * sigmoid(h[:, C:])         # GLU -> (T, C)
    w  = softmax((xg @ w_wt.T + b_wt).reshape(T, H, K), axis=-1)
    out[c, t] = sum_k xg[t+k-3, c] * w[t, h(c), k]    # depthwise dynamic conv
    y  = (out + conv_bias) @ w_out.T + b_out

Sharding: data-parallel over batch B=8 -> one batch element per NeuronCore.
Each core runs an identical program on its slice; no collectives.

Per-core dataflow (all matmuls bf16, fp32 accumulation), fully software-
pipelined in ONE loop over 32 time-tiles of 128 tokens so every engine
stays busy and the PE never sees a pool barrier:
  iter m: PE: transpose xg(m-1) -> xgT; conv(m-6) banded matmuls;
          mm_out(m-7); mm1(m); every 4th iter the dynamic-weight logits
          matmul + e^x transposes for one 512-token chunk.
          ACT: sigmoid(m), exp(chunk);  DVE: GLU mul, conv-psum copy +
          band-edge adds, y copy, token-major softmax (reduce over K,
          reciprocal, broadcast mul);  GPSIMD: band scatter(m-6);
          DMA: x chunk prefetch, wsm shift copies, y store.
  - x arrives from host pre-transposed AND pre-cast to bf16 (the PE
    consumes bf16 anyway), halving the input DMA bytes.
  - The dynamic conv is a banded matmul per (h, time-tile): out_h =
    xg_slab.T @ D where D[t', t] is a 7-diagonal band, materialized by a
    gpsimd local_scatter from shifted softmax weights; scatter indices
    are host-precomputed constants.  Cross-tile band halo is resolved by
    DVE edge adds between adjacent tiles' psum results.
  - softmax over K runs token-major on DVE (reduce/reciprocal/mul) after
    a PE transpose of exp(logits); no PE helper matmuls needed.
"""

import os
import sys

import numpy as np

for _p in ("/opt/trn_rl_repo", os.path.expanduser("~/.axon_site/_ro/trn_rl_repo")):
    if os.path.isdir(_p) and _p not in sys.path:
        sys.path.insert(0, _p)

import concourse.bacc as bacc
import concourse.bass as bass
import concourse.mybir as mybir
import concourse.tile as tile
from concourse.bass_utils import run_bass_kernel_spmd

try:
    import ml_dtypes

    BF16 = np.dtype(ml_dtypes.bfloat16)
except ImportError:  # pragma: no cover
    BF16 = None

T, B, C = 4096, 8, 512
H, K = 8, 7
PAD_L = K // 2
C2 = 2 * C
HK = H * K  # 56
P = 128

F32 = mybir.dt.float32
BF = mybir.dt.bfloat16
I16 = mybir.dt.int16

# Dt tile layout: per h a 136-wide block holding the 134 band columns of one
# 128-timestep tile (columns j <-> t = t0 + j - 3).
MAIN_W = 136
DT_W = H * MAIN_W  # 1088
CW = P + 2 * PAD_L  # 134 band columns per tile


def ts(i, size):
    return slice(i * size, (i + 1) * size)


def host_scatter_idxs():
    """Scatter index table: data element (p, i, h) -> column of the Dt tile.

    data[p, i*8+h] = wsm[t0 + p + i - 3, 7h + 6 - i]; its band column is
    j = p + i (column j of block h covers output time t0 + j - 3).
    """
    p = np.arange(P)[:, None, None]
    i = np.arange(K)[None, :, None]
    h = np.arange(H)[None, None, :]
    idx = MAIN_W * h + p + i
    return np.ascontiguousarray(idx.reshape(P, K * H).astype(np.int16))


def build_nc(t_len=T, with_bias_in=False, with_bias_wt=False, with_bias_out=False,
             with_conv_bias=False):
    """Build the single-core Bass program (shared by all 8 cores)."""
    NT = t_len // P   # time tiles of 128
    NC = t_len // 512  # 512-token chunks

    nc = bacc.Bacc()

    x_d = nc.declare_dram_parameter("xT", [C, t_len], BF, isOutput=False)
    w_inT_d = nc.declare_dram_parameter("w_inT", [P, 4, C2], BF, isOutput=False)
    w_wtT_d = nc.declare_dram_parameter("w_wtT", [P, 4, HK], BF, isOutput=False)
    w_outT_d = nc.declare_dram_parameter("w_outT", [P, 4, C], BF, isOutput=False)
    idxs_d = nc.declare_dram_parameter("idxs", [P, HK], I16, isOutput=False)
    ident16_d = nc.declare_dram_parameter("ident16", [P, P], BF, isOutput=False)
    if with_bias_in:
        b_in_d = nc.declare_dram_parameter("b_in", [C2], F32, isOutput=False)
    if with_bias_wt:
        b_wt_d = nc.declare_dram_parameter("b_wt", [HK], F32, isOutput=False)
    if with_bias_out:
        b_out_d = nc.declare_dram_parameter("b_out", [C], F32, isOutput=False)
    if with_conv_bias:
        cb4_d = nc.declare_dram_parameter("cb4", [P, 4], F32, isOutput=False)
    y_d = nc.declare_dram_parameter("y", [t_len, C], F32, isOutput=True)

    with tile.TileContext(nc) as tc:
        with (
            tc.tile_pool(name="const", bufs=1) as const,
            tc.tile_pool(name="big", bufs=1) as big,
            tc.tile_pool(name="xin", bufs=3) as xin,
            tc.tile_pool(name="xgTp", bufs=2) as xgTp,
            tc.tile_pool(name="work", bufs=2) as work,
            tc.tile_pool(name="dtp", bufs=2) as dtp,
            tc.tile_pool(name="outp", bufs=2) as outp,
            tc.tile_pool(name="ps_mm1", bufs=1,
                         space=bass.MemorySpace.PSUM) as ps_mm1,
            tc.tile_pool(name="ps_tr", bufs=2,
                         space=bass.MemorySpace.PSUM) as ps_tr,
            tc.tile_pool(name="ps_wl", bufs=1,
                         space=bass.MemorySpace.PSUM) as ps_wl,
            tc.tile_pool(name="ps_c", bufs=1,
                         space=bass.MemorySpace.PSUM) as ps_c,
            tc.tile_pool(name="ps_o", bufs=1,
                         space=bass.MemorySpace.PSUM) as ps_o,
        ):
            # ---- constants ----
            sb_winT = const.tile([P, 4, C2], BF)
            nc.sync.dma_start(sb_winT[:], w_inT_d[:])
            sb_wwtT = const.tile([P, 4, HK], BF)
            nc.sync.dma_start(sb_wwtT[:], w_wtT_d[:])
            sb_woutT = const.tile([P, 4, C], BF)
            nc.sync.dma_start(sb_woutT[:], w_outT_d[:])
            sb_idxs = const.tile([P, HK], I16)
            nc.sync.dma_start(sb_idxs[:], idxs_d[:])
            sb_id16 = const.tile([P, P], BF)
            nc.sync.dma_start(sb_id16[:], ident16_d[:])
            if with_bias_in:
                sb_bin = const.tile([P, C2], F32)
                nc.sync.dma_start(sb_bin[:], b_in_d[None, :].to_broadcast((P, C2)))
            if with_bias_wt:
                sb_bwt = const.tile([HK, 1], F32)
                nc.sync.dma_start(sb_bwt[:], b_wt_d[:, None])
            if with_bias_out:
                sb_bout = const.tile([P, C], F32)
                nc.sync.dma_start(sb_bout[:], b_out_d[None, :].to_broadcast((P, C)))
            if with_conv_bias:
                sb_cb4 = const.tile([P, 4], F32)
                nc.sync.dma_start(sb_cb4[:], cb4_d[:])

            # ---- persistent activations ----
            xg = big.tile([P, NT, C], BF)          # [t%128, t//128, c]
            conv = big.tile([P, 4, t_len], BF)     # [c%128, c//128, t]
            wsm3 = big.tile([P, K, NT, H], BF)     # [t%128, k, t//128, h]
            data_tmp = big.tile([P, K, NT, H], BF)
            data_all = big.tile([P, NT, HK], BF)

            nc.gpsimd.memset(data_tmp[:], 0.0)

            # rotating x chunks and xgT chunks (consumed within a few iters)
            x_chunks = {}
            xgT_chunks = {}

            def prefetch_chunk(c):
                # 1024-token x chunk (covers 8 time tiles), 4 DMAs
                xc = xin.tile([P, 4, 1024], BF, tag="xc", name=f"xc{c}")
                for q in range(4):
                    nc.sync.dma_start(xc[:, q, :], x_d[ts(q, P), ts(c, 1024)])
                x_chunks[c] = xc

            def mm1_glu(m):
                ps_a = ps_mm1.tile([P, C], F32, tag="ps_a")
                ps_g = ps_mm1.tile([P, C], F32, tag="ps_g")
                xc = x_chunks[m // 8]
                for q in range(4):
                    lhs = xc[:, q, ts(m % 8, P)]
                    nc.tensor.matmul(ps_a[:], lhs, sb_winT[:, q, 0:C],
                                     start=(q == 0), stop=(q == 3))
                    nc.tensor.matmul(ps_g[:], lhs, sb_winT[:, q, C:C2],
                                     start=(q == 0), stop=(q == 3))
                sig = work.tile([P, C], F32, tag="sig")
                if with_bias_in:
                    tmp_g = work.tile([P, C], F32, tag="tmp_g")
                    nc.vector.tensor_add(tmp_g[:], ps_g[:], sb_bin[:, C:C2])
                    nc.scalar.activation(sig[:], tmp_g[:],
                                         mybir.ActivationFunctionType.Sigmoid)
                    tmp_a = work.tile([P, C], F32, tag="tmp_a")
                    nc.vector.tensor_add(tmp_a[:], ps_a[:], sb_bin[:, 0:C])
                    nc.vector.tensor_mul(xg[:, m, :], tmp_a[:], sig[:])
                else:
                    nc.scalar.activation(sig[:], ps_g[:],
                                         mybir.ActivationFunctionType.Sigmoid)
                    nc.vector.tensor_mul(xg[:, m, :], ps_a[:], sig[:])

            def transpose_xg(m):
                # xg(m) -> xgT chunk slice via PE transpose + scalar copy
                c = m // 4
                if m % 4 == 0:
                    xgT_chunks[c] = xgTp.tile([P, 4, 512], BF, tag="xgT",
                                              name=f"xgT{c}")
                pxgT = ps_tr.tile([P, 4, P], BF, tag="tr")
                for q in range(4):
                    nc.tensor.transpose(pxgT[:, q, :], xg[:, m, ts(q, P)],
                                        sb_id16[:])
                nc.scalar.copy(xgT_chunks[c][:, :, ts(m % 4, P)], pxgT[:])

            e2_chunks = {}

            def weights_mm(n):
                # dynamic-weight logits for tokens [512n, 512n+512) in the
                # C-major [hk, t] domain + exp on ACT (logits are bounded, no
                # max-subtract); the transpose/normalize runs next iteration.
                xgTc = xgT_chunks[n]
                pw2 = ps_wl.tile([HK, 512], F32, tag="w1")
                for q in range(4):
                    nc.tensor.matmul(pw2[:], sb_wwtT[:, q, :], xgTc[:, q, :],
                                     start=(q == 0), stop=(q == 3))
                e2 = work.tile([HK, 512], BF, tag="e2", name=f"e2_{n}")
                if with_bias_wt:
                    nc.scalar.activation(e2[:], pw2[:],
                                         mybir.ActivationFunctionType.Exp,
                                         bias=sb_bwt[:])
                else:
                    nc.scalar.activation(e2[:], pw2[:],
                                         mybir.ActivationFunctionType.Exp)
                e2_chunks[n] = e2

            def weights_finish(n):
                # PE transpose of exp(logits) to token-major, then softmax
                # normalization on DVE: sum over K, 1/s, broadcast multiply.
                e2 = e2_chunks.pop(n)
                ptr = ps_tr.tile([P, 4, HK], BF, tag="tr", name="ptr")
                for j in range(4):
                    nc.tensor.transpose(ptr[:, j, :], e2[:, ts(j, P)],
                                        sb_id16[0:HK, 0:HK])
                pv = ptr[:].rearrange("p m (h k) -> p m h k", k=K)
                s8 = work.tile([P, 4, H], F32, tag="s8")
                nc.vector.tensor_reduce(s8[:], pv, mybir.AxisListType.X,
                                        mybir.AluOpType.add)
                r8 = work.tile([P, 4, H], F32, tag="r8")
                nc.vector.reciprocal_approx_fast(r8[:], s8[:])
                w_dst = wsm3[:, :, ts(n, 4), :].transpose([0, 2, 3, 1])
                nc.vector.tensor_tensor(
                    w_dst, pv, r8[:, :, :, None].to_broadcast((P, 4, H, K)),
                    mybir.AluOpType.mult)

            def build_dmas(mlo, mhi):
                # shifted copies of wsm3 feeding the band scatter, for time
                # tiles [mlo, mhi); issue spread over the gpsimd and sync DMA
                # queues so no single engine queue saturates
                engs = [nc.gpsimd, nc.sync]
                for i in range(K):
                    d = i - 3
                    kk = 6 - i
                    eng = engs[i % 2]
                    if d == 0:
                        eng.dma_start(data_tmp[:, i, mlo:mhi, :],
                                      wsm3[:, kk, mlo:mhi, :])
                    elif d < 0:
                        eng.dma_start(data_tmp[-d:P, i, mlo:mhi, :],
                                      wsm3[0:P + d, kk, mlo:mhi, :])
                        lo = max(mlo, 1)
                        if lo < mhi:
                            nc.sync.dma_start(data_tmp[0:-d, i, lo:mhi, :],
                                              wsm3[P + d:P, kk, lo - 1:mhi - 1, :])
                    else:
                        eng.dma_start(data_tmp[0:P - d, i, mlo:mhi, :],
                                      wsm3[d:P, kk, mlo:mhi, :])
                        hi = min(mhi, NT - 1)
                        if mlo < hi:
                            nc.sync.dma_start(data_tmp[P - d:P, i, mlo:hi, :],
                                              wsm3[0:d, kk, mlo + 1:hi + 1, :])

            def build_permute(mlo, mhi):
                # permute [p, i, m, h] -> [p, m, (i, h)]
                da4 = data_all[:, mlo:mhi, :].rearrange("p m (i h) -> p m i h",
                                                        h=H)
                nc.vector.tensor_copy(
                    da4, data_tmp[:, :, mlo:mhi, :].transpose([0, 2, 1, 3]))

            dt_tiles = {}
            el_tiles = {}

            def scatter_tile(m):
                dt = dtp.tile([P, DT_W], BF, tag="dt", name=f"dt{m}")
                nc.gpsimd.local_scatter(dt[:], data_all[:, m, :], sb_idxs[:],
                                        channels=P, num_elems=DT_W, num_idxs=HK)
                dt_tiles[m] = dt

            def conv_tile(m):
                dt = dt_tiles.pop(m)
                # [128, 4, 256] f32 = two PSUM banks; each 134-wide plane pair
                # stays inside a single bank
                pc = ps_c.tile([P, 4, 256], F32, tag="pc", name=f"pc{m}")
                pcv = pc[:, :, 0:CW]
                for ci in range(4):
                    for hp, pb in ((0, 0), (1, 64)):
                        hh = ci * 2 + hp
                        nc.tensor.matmul(
                            pcv[pb:pb + 64, ci, :], xg[:, m, ts(hh, 64)],
                            dt[:, MAIN_W * hh:MAIN_W * hh + CW],
                            start=True, stop=True, skip_group_check=True)
                t0 = m * P
                if m >= 1:
                    # right edge of tile m-1 first: it unblocks mm_out(m-1)
                    dr = conv[:, :, t0 - PAD_L:t0]
                    nc.vector.tensor_add(dr, dr, pcv[:, :, 0:PAD_L])
                # body of tile m (must precede its left-edge add)
                if with_conv_bias:
                    for ci in range(4):
                        nc.vector.tensor_scalar_add(
                            conv[:, ci, t0:t0 + P], pcv[:, ci, PAD_L:PAD_L + P],
                            sb_cb4[:, ci:ci + 1])
                else:
                    nc.vector.tensor_copy(conv[:, :, t0:t0 + P],
                                          pcv[:, :, PAD_L:PAD_L + P])
                if m - 1 in el_tiles:
                    # left edge of tile m: tile m-1 rows feeding t0..t0+2
                    dl = conv[:, :, t0:t0 + PAD_L]
                    nc.vector.tensor_add(dl, dl, el_tiles.pop(m - 1)[:])
                if m + 1 < NT:
                    # stage the outgoing right-edge so pc needs one generation
                    el = work.tile([P, 4, PAD_L], F32, tag="el", name=f"el{m}")
                    nc.vector.tensor_copy(el[:], pcv[:, :, CW - PAD_L:CW])
                    el_tiles[m] = el

            def mm_out(m):
                po = ps_o.tile([P, C], F32, tag="po")
                for q in range(4):
                    nc.tensor.matmul(po[:], conv[:, q, ts(m, P)],
                                     sb_woutT[:, q, :],
                                     start=(q == 0), stop=(q == 3))
                out_t = outp.tile([P, C], F32, tag="out_t")
                if with_bias_out:
                    nc.vector.tensor_add(out_t[:], po[:], sb_bout[:])
                else:
                    nc.scalar.copy(out_t[:], po[:])
                nc.sync.dma_start(y_d[ts(m, P), :], out_t[:])

            # ---- software-pipelined main loop ----
            # iter m: softmax-finish + build batch (every 4th) | conv(m-8) |
            #         tr(m-1) | scatter(m-7) | mm1(m) | logits chunk (every
            #         4th, before mm_out so exp hides under the iter tail) |
            #         mm_out(m-9)
            prefetch_chunk(0)
            SC_LAG, CONV_LAG, OUT_LAG = 7, 8, 9
            for m in range(NT + OUT_LAG + 1):
                if m % 8 == 1 and m // 8 + 1 < t_len // 1024:
                    prefetch_chunk(m // 8 + 1)
                bb = None
                if m % 4 == 1 and m >= 5:
                    # batch n covers tiles [4n-1, 4n+3); the last batch
                    # (n == NC) covers just the final tile
                    n = (m - 5) // 4
                    if n < NC:
                        weights_finish(n)
                    bb = (max(4 * n - 1, 0), min(4 * n + 3, NT))
                    if bb[0] < bb[1]:
                        build_dmas(*bb)
                    else:
                        bb = None
                if CONV_LAG <= m < NT + CONV_LAG:
                    conv_tile(m - CONV_LAG)
                if bb is not None:
                    build_permute(*bb)
                if 1 <= m <= NT:
                    transpose_xg(m - 1)
                if SC_LAG <= m < NT + SC_LAG:
                    scatter_tile(m - SC_LAG)
                if m < NT:
                    mm1_glu(m)
                if m % 4 == 0 and 4 <= m <= NT:
                    weights_mm(m // 4 - 1)
                if OUT_LAG <= m <= NT - 1 + OUT_LAG:
                    mm_out(m - OUT_LAG)

    nc.compile()
    return nc


def host_inputs(x_b, w_in, b_in, w_wt, b_wt, w_out, b_out, conv_bias,
                with_bias_in, with_bias_wt, with_bias_out, with_conv_bias):
    """Per-core input map from a batch slice + shared weights."""
    def t_pack(w, width, dt_=None):
        # w: [width, C] -> [128, 4, width] with [p, q, f] = w[f, 128q+p]
        a = np.ascontiguousarray(
            w.T.reshape(4, P, width).transpose(1, 0, 2)).astype(dt_ or BF16)
        return a

    m = {
        "xT": np.ascontiguousarray(np.asarray(x_b, np.float32).T).astype(BF16),
        "w_inT": t_pack(w_in, C2),
        "w_wtT": t_pack(w_wt, HK),
        "w_outT": t_pack(w_out, C),
        "idxs": host_scatter_idxs(),
        "ident16": np.eye(P).astype(BF16),
    }
    if with_bias_in:
        m["b_in"] = np.asarray(b_in, np.float32)
    if with_bias_wt:
        m["b_wt"] = np.asarray(b_wt, np.float32)
    if with_bias_out:
        m["b_out"] = np.asarray(b_out, np.float32)
    if with_conv_bias:
        m["cb4"] = np.ascontiguousarray(
            np.asarray(conv_bias, np.float32).reshape(4, P).T)
    return m


_NC_CACHE = {}


def _get_nc(key):
    if key not in _NC_CACHE:
        _NC_CACHE[key] = build_nc(T, *key)
    return _NC_CACHE[key]


def kernel(x, w_in, b_in, w_wt, b_wt, w_out, b_out, conv_bias, _trace=False):
    x = np.asarray(x)
    flags = (bool(np.any(b_in)), bool(np.any(b_wt)), bool(np.any(b_out)),
             bool(np.any(conv_bias)))
    nc = _get_nc(flags)
    in_maps = [
        host_inputs(x[:, b, :], np.asarray(w_in), b_in, np.asarray(w_wt), b_wt,
                    np.asarray(w_out), b_out, conv_bias, *flags)
        for b in range(B)
    ]
    res = run_bass_kernel_spmd(nc, in_maps, core_ids=list(range(B)),
                               trace=_trace)
    y = np.stack([np.asarray(res.results[b]["y"]) for b in range(B)], axis=1)
    if _trace:
        return y.astype(np.float32), res
    return y.astype(np.float32)


# revision 10
# speedup vs baseline: 1.0184x; 1.0184x over previous
"""Trainium2 Bass kernel for nn_DynamicConvolution.

Reference computation (per batch b, T=4096 timesteps, C=512 channels):
    h  = x @ w_in.T + b_in                    # (T, 2C)
    xg = h[:, :C] * sigmoid(h[:, C:])         # GLU -> (T, C)
    w  = softmax((xg @ w_wt.T + b_wt).reshape(T, H, K), axis=-1)
    out[c, t] = sum_k xg[t+k-3, c] * w[t, h(c), k]    # depthwise dynamic conv
    y  = (out + conv_bias) @ w_out.T + b_out

Sharding: data-parallel over batch B=8 -> one batch element per NeuronCore.
Each core runs an identical program on its slice; no collectives.

Per-core dataflow (all matmuls bf16, fp32 accumulation), fully software-
pipelined in ONE loop over 32 time-tiles of 128 tokens so every engine
stays busy and the PE never sees a pool barrier:
  iter m: PE: transpose xg(m-1) -> xgT; conv(m-6) banded matmuls;
          mm_out(m-7); mm1(m); every 4th iter the dynamic-weight logits
          matmul + e^x transposes for one 512-token chunk.
          ACT: sigmoid(m), exp(chunk);  DVE: GLU mul, conv-psum copy +
          band-edge adds, y copy, token-major softmax (reduce over K,
          reciprocal, broadcast mul);  GPSIMD: band scatter(m-6);
          DMA: x chunk prefetch, wsm shift copies, y store.
  - x arrives from host pre-transposed AND pre-cast to bf16 (the PE
    consumes bf16 anyway), halving the input DMA bytes.
  - The dynamic conv is a banded matmul per (h, time-tile): out_h =
    xg_slab.T @ D where D[t', t] is a 7-diagonal band, materialized by a
    gpsimd local_scatter from shifted softmax weights; scatter indices
    are host-precomputed constants.  Cross-tile band halo is resolved by
    DVE edge adds between adjacent tiles' psum results.
  - softmax over K runs token-major on DVE (reduce/reciprocal/mul) after
    a PE transpose of exp(logits); no PE helper matmuls needed.
"""

import os
import sys

import numpy as np

for _p in ("/opt/trn_rl_repo", os.path.expanduser("~/.axon_site/_ro/trn_rl_repo")):
    if os.path.isdir(_p) and _p not in sys.path:
        sys.path.insert(0, _p)

import concourse.bacc as bacc
import concourse.bass as bass
import concourse.mybir as mybir
import concourse.tile as tile
from concourse.bass_utils import run_bass_kernel_spmd

try:
    import ml_dtypes

    BF16 = np.dtype(ml_dtypes.bfloat16)
except ImportError:  # pragma: no cover
    BF16 = None

T, B, C = 4096, 8, 512
H, K = 8, 7
PAD_L = K // 2
C2 = 2 * C
HK = H * K  # 56
P = 128

F32 = mybir.dt.float32
BF = mybir.dt.bfloat16
I16 = mybir.dt.int16

# Dt tile layout: per h a 136-wide block holding the 134 band columns of one
# 128-timestep tile (columns j <-> t = t0 + j - 3).
MAIN_W = 136
DT_W = H * MAIN_W  # 1088
CW = P + 2 * PAD_L  # 134 band columns per tile


def ts(i, size):
    return slice(i * size, (i + 1) * size)


def host_scatter_idxs():
    """Scatter index table: data element (p, i, h) -> column of the Dt tile.

    data[p, i*8+h] = wsm[t0 + p + i - 3, 7h + 6 - i]; its band column is
    j = p + i (column j of block h covers output time t0 + j - 3).
    """
    p = np.arange(P)[:, None, None]
    i = np.arange(K)[None, :, None]
    h = np.arange(H)[None, None, :]
    idx = MAIN_W * h + p + i
    return np.ascontiguousarray(idx.reshape(P, K * H).astype(np.int16))


def build_nc(t_len=T, with_bias_in=False, with_bias_wt=False, with_bias_out=False,
             with_conv_bias=False):
    """Build the single-core Bass program (shared by all 8 cores)."""
    NT = t_len // P   # time tiles of 128
    NC = t_len // 512  # 512-token chunks

    nc = bacc.Bacc()

    x_d = nc.declare_dram_parameter("xT", [C, t_len], BF, isOutput=False)
    w_inT_d = nc.declare_dram_parameter("w_inT", [P, 4, C2], BF, isOutput=False)
    w_wtT_d = nc.declare_dram_parameter("w_wtT", [P, 4, HK], BF, isOutput=False)
    w_outT_d = nc.declare_dram_parameter("w_outT", [P, 4, C], BF, isOutput=False)
    idxs_d = nc.declare_dram_parameter("idxs", [P, HK], I16, isOutput=False)
    ident16_d = nc.declare_dram_parameter("ident16", [P, P], BF, isOutput=False)
    if with_bias_in:
        b_in_d = nc.declare_dram_parameter("b_in", [C2], F32, isOutput=False)
    if with_bias_wt:
        b_wt_d = nc.declare_dram_parameter("b_wt", [HK], F32, isOutput=False)
    if with_bias_out:
        b_out_d = nc.declare_dram_parameter("b_out", [C], F32, isOutput=False)
    if with_conv_bias:
        cb4_d = nc.declare_dram_parameter("cb4", [P, 4], F32, isOutput=False)
    y_d = nc.declare_dram_parameter("y", [t_len, C], F32, isOutput=True)

    with tile.TileContext(nc) as tc:
        with (
            tc.tile_pool(name="const", bufs=1) as const,
            tc.tile_pool(name="big", bufs=1) as big,
            tc.tile_pool(name="xin", bufs=3) as xin,
            tc.tile_pool(name="xgTp", bufs=2) as xgTp,
            tc.tile_pool(name="work", bufs=2) as work,
            tc.tile_pool(name="dtp", bufs=2) as dtp,
            tc.tile_pool(name="outp", bufs=2) as outp,
            tc.tile_pool(name="ps_mm1", bufs=1,
                         space=bass.MemorySpace.PSUM) as ps_mm1,
            tc.tile_pool(name="ps_tr", bufs=2,
                         space=bass.MemorySpace.PSUM) as ps_tr,
            tc.tile_pool(name="ps_wl", bufs=1,
                         space=bass.MemorySpace.PSUM) as ps_wl,
            tc.tile_pool(name="ps_c", bufs=1,
                         space=bass.MemorySpace.PSUM) as ps_c,
            tc.tile_pool(name="ps_o", bufs=1,
                         space=bass.MemorySpace.PSUM) as ps_o,
        ):
            # ---- constants ----
            sb_winT = const.tile([P, 4, C2], BF)
            nc.sync.dma_start(sb_winT[:], w_inT_d[:])
            sb_wwtT = const.tile([P, 4, HK], BF)
            nc.sync.dma_start(sb_wwtT[:], w_wtT_d[:])
            sb_woutT = const.tile([P, 4, C], BF)
            nc.sync.dma_start(sb_woutT[:], w_outT_d[:])
            sb_idxs = const.tile([P, HK], I16)
            nc.sync.dma_start(sb_idxs[:], idxs_d[:])
            sb_id16 = const.tile([P, P], BF)
            nc.sync.dma_start(sb_id16[:], ident16_d[:])
            if with_bias_in:
                sb_bin = const.tile([P, C2], F32)
                nc.sync.dma_start(sb_bin[:], b_in_d[None, :].to_broadcast((P, C2)))
            if with_bias_wt:
                sb_bwt = const.tile([HK, 1], F32)
                nc.sync.dma_start(sb_bwt[:], b_wt_d[:, None])
            if with_bias_out:
                sb_bout = const.tile([P, C], F32)
                nc.sync.dma_start(sb_bout[:], b_out_d[None, :].to_broadcast((P, C)))
            if with_conv_bias:
                sb_cb4 = const.tile([P, 4], F32)
                nc.sync.dma_start(sb_cb4[:], cb4_d[:])

            # ---- persistent activations ----
            xg = big.tile([P, NT, C], BF)          # [t%128, t//128, c]
            conv = big.tile([P, 4, t_len], BF)     # [c%128, c//128, t]
            wsm3 = big.tile([P, K, NT, H], BF)     # [t%128, k, t//128, h]
            data_tmp = big.tile([P, K, NT, H], BF)
            data_all = big.tile([P, NT, HK], BF)

            nc.gpsimd.memset(data_tmp[:], 0.0)

            # rotating x chunks and xgT chunks (consumed within a few iters)
            x_chunks = {}
            xgT_chunks = {}

            def prefetch_chunk(c):
                # 1024-token x chunk (covers 8 time tiles), 4 DMAs
                xc = xin.tile([P, 4, 1024], BF, tag="xc", name=f"xc{c}")
                for q in range(4):
                    nc.sync.dma_start(xc[:, q, :], x_d[ts(q, P), ts(c, 1024)])
                x_chunks[c] = xc

            def mm1_glu(m):
                ps_a = ps_mm1.tile([P, C], F32, tag="ps_a")
                ps_g = ps_mm1.tile([P, C], F32, tag="ps_g")
                xc = x_chunks[m // 8]
                for q in range(4):
                    lhs = xc[:, q, ts(m % 8, P)]
                    nc.tensor.matmul(ps_a[:], lhs, sb_winT[:, q, 0:C],
                                     start=(q == 0), stop=(q == 3))
                    nc.tensor.matmul(ps_g[:], lhs, sb_winT[:, q, C:C2],
                                     start=(q == 0), stop=(q == 3))
                sig = work.tile([P, C], F32, tag="sig")
                if with_bias_in:
                    tmp_g = work.tile([P, C], F32, tag="tmp_g")
                    nc.vector.tensor_add(tmp_g[:], ps_g[:], sb_bin[:, C:C2])
                    nc.scalar.activation(sig[:], tmp_g[:],
                                         mybir.ActivationFunctionType.Sigmoid)
                    tmp_a = work.tile([P, C], F32, tag="tmp_a")
                    nc.vector.tensor_add(tmp_a[:], ps_a[:], sb_bin[:, 0:C])
                    nc.vector.tensor_mul(xg[:, m, :], tmp_a[:], sig[:])
                else:
                    nc.scalar.activation(sig[:], ps_g[:],
                                         mybir.ActivationFunctionType.Sigmoid)
                    nc.vector.tensor_mul(xg[:, m, :], ps_a[:], sig[:])

            def transpose_xg(m):
                # xg(m) -> xgT chunk slice via PE transpose + scalar copy
                c = m // 4
                if m % 4 == 0:
                    xgT_chunks[c] = xgTp.tile([P, 4, 512], BF, tag="xgT",
                                              name=f"xgT{c}")
                pxgT = ps_tr.tile([P, 4, P], BF, tag="tr")
                for q in range(4):
                    nc.tensor.transpose(pxgT[:, q, :], xg[:, m, ts(q, P)],
                                        sb_id16[:])
                nc.scalar.copy(xgT_chunks[c][:, :, ts(m % 4, P)], pxgT[:])

            e2_chunks = {}

            def weights_mm(n):
                # dynamic-weight logits for tokens [512n, 512n+512) in the
                # C-major [hk, t] domain + exp on ACT (logits are bounded, no
                # max-subtract); the transpose/normalize runs next iteration.
                xgTc = xgT_chunks[n]
                pw2 = ps_wl.tile([HK, 512], F32, tag="w1")
                for q in range(4):
                    nc.tensor.matmul(pw2[:], sb_wwtT[:, q, :], xgTc[:, q, :],
                                     start=(q == 0), stop=(q == 3))
                e2 = work.tile([HK, 512], BF, tag="e2", name=f"e2_{n}")
                if with_bias_wt:
                    nc.scalar.activation(e2[:], pw2[:],
                                         mybir.ActivationFunctionType.Exp,
                                         bias=sb_bwt[:])
                else:
                    nc.scalar.activation(e2[:], pw2[:],
                                         mybir.ActivationFunctionType.Exp)
                e2_chunks[n] = e2

            def weights_finish(n):
                # PE transpose of exp(logits) to token-major, then softmax
                # normalization on DVE: sum over K, 1/s, broadcast multiply.
                e2 = e2_chunks.pop(n)
                ptr = ps_tr.tile([P, 4, HK], BF, tag="tr", name="ptr")
                for j in range(4):
                    nc.tensor.transpose(ptr[:, j, :], e2[:, ts(j, P)],
                                        sb_id16[0:HK, 0:HK])
                pv = ptr[:].rearrange("p m (h k) -> p m h k", k=K)
                s8 = work.tile([P, 4, H], F32, tag="s8")
                nc.vector.tensor_reduce(s8[:], pv, mybir.AxisListType.X,
                                        mybir.AluOpType.add)
                r8 = work.tile([P, 4, H], F32, tag="r8")
                nc.vector.reciprocal_approx_fast(r8[:], s8[:])
                w_dst = wsm3[:, :, ts(n, 4), :].transpose([0, 2, 3, 1])
                nc.vector.tensor_tensor(
                    w_dst, pv, r8[:, :, :, None].to_broadcast((P, 4, H, K)),
                    mybir.AluOpType.mult)

            def build_dmas(mlo, mhi):
                # shifted copies of wsm3 feeding the band scatter, for time
                # tiles [mlo, mhi); main copies issue from sync (HWDGE), halo
                # wrap copies from vector, keeping gpsimd scatter-only
                for i in range(K):
                    d = i - 3
                    kk = 6 - i
                    if d == 0:
                        nc.sync.dma_start(data_tmp[:, i, mlo:mhi, :],
                                          wsm3[:, kk, mlo:mhi, :])
                    elif d < 0:
                        nc.sync.dma_start(data_tmp[-d:P, i, mlo:mhi, :],
                                          wsm3[0:P + d, kk, mlo:mhi, :])
                        lo = max(mlo, 1)
                        if lo < mhi:
                            nc.scalar.dma_start(data_tmp[0:-d, i, lo:mhi, :],
                                                wsm3[P + d:P, kk, lo - 1:mhi - 1, :])
                    else:
                        nc.sync.dma_start(data_tmp[0:P - d, i, mlo:mhi, :],
                                          wsm3[d:P, kk, mlo:mhi, :])
                        hi = min(mhi, NT - 1)
                        if mlo < hi:
                            nc.scalar.dma_start(data_tmp[P - d:P, i, mlo:hi, :],
                                                wsm3[0:d, kk, mlo + 1:hi + 1, :])

            def build_permute(mlo, mhi):
                # permute [p, i, m, h] -> [p, m, (i, h)]
                da4 = data_all[:, mlo:mhi, :].rearrange("p m (i h) -> p m i h",
                                                        h=H)
                nc.vector.tensor_copy(
                    da4, data_tmp[:, :, mlo:mhi, :].transpose([0, 2, 1, 3]))

            dt_tiles = {}
            el_tiles = {}

            def scatter_tile(m):
                dt = dtp.tile([P, DT_W], BF, tag="dt", name=f"dt{m}")
                nc.gpsimd.local_scatter(dt[:], data_all[:, m, :], sb_idxs[:],
                                        channels=P, num_elems=DT_W, num_idxs=HK)
                dt_tiles[m] = dt

            def conv_tile(m):
                dt = dt_tiles.pop(m)
                # [128, 4, 256] f32 = two PSUM banks; each 134-wide plane pair
                # stays inside a single bank
                pc = ps_c.tile([P, 4, 256], F32, tag="pc", name=f"pc{m}")
                pcv = pc[:, :, 0:CW]
                for ci in range(4):
                    for hp, pb in ((0, 0), (1, 64)):
                        hh = ci * 2 + hp
                        nc.tensor.matmul(
                            pcv[pb:pb + 64, ci, :], xg[:, m, ts(hh, 64)],
                            dt[:, MAIN_W * hh:MAIN_W * hh + CW],
                            start=True, stop=True, skip_group_check=True)
                t0 = m * P
                if m >= 1:
                    # right edge of tile m-1 first: it unblocks mm_out(m-1)
                    dr = conv[:, :, t0 - PAD_L:t0]
                    nc.vector.tensor_add(dr, dr, pcv[:, :, 0:PAD_L])
                # body of tile m (must precede its left-edge add)
                if with_conv_bias:
                    for ci in range(4):
                        nc.vector.tensor_scalar_add(
                            conv[:, ci, t0:t0 + P], pcv[:, ci, PAD_L:PAD_L + P],
                            sb_cb4[:, ci:ci + 1])
                else:
                    nc.vector.tensor_copy(conv[:, :, t0:t0 + P],
                                          pcv[:, :, PAD_L:PAD_L + P])
                if m - 1 in el_tiles:
                    # left edge of tile m: tile m-1 rows feeding t0..t0+2
                    dl = conv[:, :, t0:t0 + PAD_L]
                    nc.vector.tensor_add(dl, dl, el_tiles.pop(m - 1)[:])
                if m + 1 < NT:
                    # stage the outgoing right-edge so pc needs one generation
                    el = work.tile([P, 4, PAD_L], F32, tag="el", name=f"el{m}")
                    nc.vector.tensor_copy(el[:], pcv[:, :, CW - PAD_L:CW])
                    el_tiles[m] = el

            def mm_out(m):
                po = ps_o.tile([P, C], F32, tag="po")
                for q in range(4):
                    nc.tensor.matmul(po[:], conv[:, q, ts(m, P)],
                                     sb_woutT[:, q, :],
                                     start=(q == 0), stop=(q == 3))
                out_t = outp.tile([P, C], F32, tag="out_t")
                if with_bias_out:
                    nc.vector.tensor_add(out_t[:], po[:], sb_bout[:])
                else:
                    nc.vector.tensor_copy(out_t[:], po[:])
                nc.sync.dma_start(y_d[ts(m, P), :], out_t[:])

            # ---- software-pipelined main loop ----
            # iter m: softmax-finish + build batch (every 4th) | conv(m-8) |
            #         tr(m-1) | scatter(m-7) | mm1(m) | logits chunk (every
            #         4th, before mm_out so exp hides under the iter tail) |
            #         mm_out(m-9)
            prefetch_chunk(0)
            SC_LAG, CONV_LAG, OUT_LAG = 9, 10, 11
            for m in range(NT + OUT_LAG + 1):
                if m % 8 == 1 and m // 8 + 1 < t_len // 1024:
                    prefetch_chunk(m // 8 + 1)
                bb = None
                if m % 4 == 1 and m >= 5:
                    # batch n covers tiles [4n-1, 4n+3); the last batch
                    # (n == NC) covers just the final tile
                    n = (m - 5) // 4
                    if n < NC:
                        weights_finish(n)
                    bb = (max(4 * n - 1, 0), min(4 * n + 3, NT))
                    if bb[0] < bb[1]:
                        build_dmas(*bb)
                    else:
                        bb = None
                if CONV_LAG <= m < NT + CONV_LAG:
                    conv_tile(m - CONV_LAG)
                if bb is not None:
                    build_permute(*bb)
                if 1 <= m <= NT:
                    transpose_xg(m - 1)
                if SC_LAG <= m < NT + SC_LAG:
                    scatter_tile(m - SC_LAG)
                if m < NT:
                    mm1_glu(m)
                if m % 4 == 0 and 4 <= m <= NT:
                    weights_mm(m // 4 - 1)
                if OUT_LAG <= m <= NT - 1 + OUT_LAG:
                    mm_out(m - OUT_LAG)

    nc.compile()
    return nc


def host_inputs(x_b, w_in, b_in, w_wt, b_wt, w_out, b_out, conv_bias,
                with_bias_in, with_bias_wt, with_bias_out, with_conv_bias):
    """Per-core input map from a batch slice + shared weights."""
    def t_pack(w, width, dt_=None):
        # w: [width, C] -> [128, 4, width] with [p, q, f] = w[f, 128q+p]
        a = np.ascontiguousarray(
            w.T.reshape(4, P, width).transpose(1, 0, 2)).astype(dt_ or BF16)
        return a

    m = {
        "xT": np.ascontiguousarray(np.asarray(x_b, np.float32).T).astype(BF16),
        "w_inT": t_pack(w_in, C2),
        "w_wtT": t_pack(w_wt, HK),
        "w_outT": t_pack(w_out, C),
        "idxs": host_scatter_idxs(),
        "ident16": np.eye(P).astype(BF16),
    }
    if with_bias_in:
        m["b_in"] = np.asarray(b_in, np.float32)
    if with_bias_wt:
        m["b_wt"] = np.asarray(b_wt, np.float32)
    if with_bias_out:
        m["b_out"] = np.asarray(b_out, np.float32)
    if with_conv_bias:
        m["cb4"] = np.ascontiguousarray(
            np.asarray(conv_bias, np.float32).reshape(4, P).T)
    return m


_NC_CACHE = {}


def _get_nc(key):
    if key not in _NC_CACHE:
        _NC_CACHE[key] = build_nc(T, *key)
    return _NC_CACHE[key]


def kernel(x, w_in, b_in, w_wt, b_wt, w_out, b_out, conv_bias, _trace=False):
    x = np.asarray(x)
    flags = (bool(np.any(b_in)), bool(np.any(b_wt)), bool(np.any(b_out)),
             bool(np.any(conv_bias)))
    nc = _get_nc(flags)
    in_maps = [
        host_inputs(x[:, b, :], np.asarray(w_in), b_in, np.asarray(w_wt), b_wt,
                    np.asarray(w_out), b_out, conv_bias, *flags)
        for b in range(B)
    ]
    res = run_bass_kernel_spmd(nc, in_maps, core_ids=list(range(B)),
                               trace=_trace)
    y = np.stack([np.asarray(res.results[b]["y"]) for b in range(B)], axis=1)
    if _trace:
        return y.astype(np.float32), res
    return y.astype(np.float32)


# revision 11
# speedup vs baseline: 1.1220x; 1.1017x over previous
"""Trainium2 Bass kernel for nn_DynamicConvolution.

Reference computation (per batch b, T=4096 timesteps, C=512 channels):
    h  = x @ w_in.T + b_in                    # (T, 2C)
    xg = h[:, :C] * sigmoid(h[:, C:])         # GLU -> (T, C)
    w  = softmax((xg @ w_wt.T + b_wt).reshape(T, H, K), axis=-1)
    out[c, t] = sum_k xg[t+k-3, c] * w[t, h(c), k]    # depthwise dynamic conv
    y  = (out + conv_bias) @ w_out.T + b_out

Sharding: data-parallel over batch B=8 -> one batch element per NeuronCore.
Each core runs an identical program on its slice; no collectives.

Per-core dataflow (all matmuls bf16, fp32 accumulation), fully software-
pipelined in ONE loop over 32 time-tiles of 128 tokens so every engine
stays busy and the PE never sees a pool barrier:
  iter m: PE: transpose xg(m-1) -> xgT; conv(m-6) banded matmuls;
          mm_out(m-7); mm1(m); every 4th iter the dynamic-weight logits
          matmul + e^x transposes for one 512-token chunk.
          ACT: sigmoid(m), exp(chunk);  DVE: GLU mul, conv-psum copy +
          band-edge adds, y copy, token-major softmax (reduce over K,
          reciprocal, broadcast mul);  GPSIMD: band scatter(m-6);
          DMA: x chunk prefetch, wsm shift copies, y store.
  - x arrives from host pre-transposed AND pre-cast to bf16 (the PE
    consumes bf16 anyway), halving the input DMA bytes.
  - The dynamic conv is a banded matmul per (h, time-tile): out_h =
    xg_slab.T @ D where D[t', t] is a 7-diagonal band, materialized by a
    gpsimd local_scatter from shifted softmax weights; scatter indices
    are host-precomputed constants.  Cross-tile band halo is resolved by
    DVE edge adds between adjacent tiles' psum results.
  - softmax over K runs token-major on DVE (reduce/reciprocal/mul) after
    a PE transpose of exp(logits); no PE helper matmuls needed.
"""

import os
import sys

import numpy as np

for _p in ("/opt/trn_rl_repo", os.path.expanduser("~/.axon_site/_ro/trn_rl_repo")):
    if os.path.isdir(_p) and _p not in sys.path:
        sys.path.insert(0, _p)

import concourse.bacc as bacc
import concourse.bass as bass
import concourse.mybir as mybir
import concourse.tile as tile
from concourse.bass_utils import run_bass_kernel_spmd

try:
    import ml_dtypes

    BF16 = np.dtype(ml_dtypes.bfloat16)
except ImportError:  # pragma: no cover
    BF16 = None

T, B, C = 4096, 8, 512
H, K = 8, 7
PAD_L = K // 2
C2 = 2 * C
HK = H * K  # 56
P = 128

F32 = mybir.dt.float32
BF = mybir.dt.bfloat16
I16 = mybir.dt.int16

# Dt tile layout: per h a 136-wide block holding the 134 band columns of one
# 128-timestep tile (columns j <-> t = t0 + j - 3).
MAIN_W = 136
DT_W = H * MAIN_W  # 1088
CW = P + 2 * PAD_L  # 134 band columns per tile


def ts(i, size):
    return slice(i * size, (i + 1) * size)


def host_scatter_idxs():
    """Scatter index table: data element (p, i, h) -> column of the Dt tile.

    data[p, i*8+h] = wsm[t0 + p + i - 3, 7h + 6 - i]; its band column is
    j = p + i (column j of block h covers output time t0 + j - 3).
    """
    p = np.arange(P)[:, None, None]
    i = np.arange(K)[None, :, None]
    h = np.arange(H)[None, None, :]
    idx = MAIN_W * h + p + i
    return np.ascontiguousarray(idx.reshape(P, K * H).astype(np.int16))


def build_nc(t_len=T, with_bias_in=False, with_bias_wt=False, with_bias_out=False,
             with_conv_bias=False):
    """Build the single-core Bass program (shared by all 8 cores)."""
    NT = t_len // P   # time tiles of 128
    NC = t_len // 512  # 512-token chunks

    nc = bacc.Bacc()

    x_d = nc.declare_dram_parameter("xT", [C, t_len], BF, isOutput=False)
    w_inT_d = nc.declare_dram_parameter("w_inT", [P, 4, C2], BF, isOutput=False)
    w_wtT_d = nc.declare_dram_parameter("w_wtT", [P, 4, HK], BF, isOutput=False)
    w_outT_d = nc.declare_dram_parameter("w_outT", [P, 4, C], BF, isOutput=False)
    idxs_d = nc.declare_dram_parameter("idxs", [P, HK], I16, isOutput=False)
    ident16_d = nc.declare_dram_parameter("ident16", [P, P], BF, isOutput=False)
    if with_bias_in:
        b_in_d = nc.declare_dram_parameter("b_in", [C2], F32, isOutput=False)
    if with_bias_wt:
        b_wt_d = nc.declare_dram_parameter("b_wt", [HK], F32, isOutput=False)
    if with_bias_out:
        b_out_d = nc.declare_dram_parameter("b_out", [C], F32, isOutput=False)
    if with_conv_bias:
        cb4_d = nc.declare_dram_parameter("cb4", [P, 4], F32, isOutput=False)
    y_d = nc.declare_dram_parameter("y", [t_len, C], F32, isOutput=True)

    with tile.TileContext(nc) as tc:
        with (
            tc.tile_pool(name="const", bufs=1) as const,
            tc.tile_pool(name="big", bufs=1) as big,
            tc.tile_pool(name="xin", bufs=3) as xin,
            tc.tile_pool(name="xgTp", bufs=2) as xgTp,
            tc.tile_pool(name="work", bufs=2) as work,
            tc.tile_pool(name="dtp", bufs=2) as dtp,
            tc.tile_pool(name="outp", bufs=2) as outp,
            tc.tile_pool(name="ps_mm1", bufs=1,
                         space=bass.MemorySpace.PSUM) as ps_mm1,
            tc.tile_pool(name="ps_tr", bufs=2,
                         space=bass.MemorySpace.PSUM) as ps_tr,
            tc.tile_pool(name="ps_wl", bufs=1,
                         space=bass.MemorySpace.PSUM) as ps_wl,
            tc.tile_pool(name="ps_c", bufs=1,
                         space=bass.MemorySpace.PSUM) as ps_c,
            tc.tile_pool(name="ps_o", bufs=1,
                         space=bass.MemorySpace.PSUM) as ps_o,
        ):
            # ---- constants ----
            sb_winT = const.tile([P, 4, C2], BF)
            nc.sync.dma_start(sb_winT[:], w_inT_d[:])
            sb_wwtT = const.tile([P, 4, HK], BF)
            nc.sync.dma_start(sb_wwtT[:], w_wtT_d[:])
            sb_woutT = const.tile([P, 4, C], BF)
            nc.sync.dma_start(sb_woutT[:], w_outT_d[:])
            sb_idxs = const.tile([P, HK], I16)
            nc.sync.dma_start(sb_idxs[:], idxs_d[:])
            sb_id16 = const.tile([P, P], BF)
            nc.sync.dma_start(sb_id16[:], ident16_d[:])
            if with_bias_in:
                sb_bin = const.tile([P, C2], F32)
                nc.sync.dma_start(sb_bin[:], b_in_d[None, :].to_broadcast((P, C2)))
            if with_bias_wt:
                sb_bwt = const.tile([HK, 1], F32)
                nc.sync.dma_start(sb_bwt[:], b_wt_d[:, None])
            if with_bias_out:
                sb_bout = const.tile([P, C], F32)
                nc.sync.dma_start(sb_bout[:], b_out_d[None, :].to_broadcast((P, C)))
            if with_conv_bias:
                sb_cb4 = const.tile([P, 4], F32)
                nc.sync.dma_start(sb_cb4[:], cb4_d[:])

            # ---- persistent activations ----
            xg = big.tile([P, NT, C], BF)          # [t%128, t//128, c]
            conv = big.tile([P, 4, t_len], BF)     # [c%128, c//128, t]
            wsm3 = big.tile([P, K, NT, H], BF)     # [t%128, k, t//128, h]
            data_tmp = big.tile([P, K, NT, H], BF)
            data_all = big.tile([P, NT, HK], BF)

            nc.gpsimd.memset(data_tmp[:], 0.0)

            # rotating x chunks and xgT chunks (consumed within a few iters)
            x_chunks = {}
            xgT_chunks = {}

            def prefetch_chunk(c):
                # 1024-token x chunk (covers 8 time tiles), 4 DMAs
                xc = xin.tile([P, 4, 1024], BF, tag="xc", name=f"xc{c}")
                for q in range(4):
                    nc.gpsimd.dma_start(xc[:, q, :], x_d[ts(q, P), ts(c, 1024)])
                x_chunks[c] = xc

            def mm1_glu(m):
                ps_a = ps_mm1.tile([P, C], F32, tag="ps_a")
                ps_g = ps_mm1.tile([P, C], F32, tag="ps_g")
                xc = x_chunks[m // 8]
                for q in range(4):
                    lhs = xc[:, q, ts(m % 8, P)]
                    nc.tensor.matmul(ps_a[:], lhs, sb_winT[:, q, 0:C],
                                     start=(q == 0), stop=(q == 3))
                    nc.tensor.matmul(ps_g[:], lhs, sb_winT[:, q, C:C2],
                                     start=(q == 0), stop=(q == 3))
                sig = work.tile([P, C], F32, tag="sig")
                if with_bias_in:
                    tmp_g = work.tile([P, C], F32, tag="tmp_g")
                    nc.vector.tensor_add(tmp_g[:], ps_g[:], sb_bin[:, C:C2])
                    nc.scalar.activation(sig[:], tmp_g[:],
                                         mybir.ActivationFunctionType.Sigmoid)
                    tmp_a = work.tile([P, C], F32, tag="tmp_a")
                    nc.vector.tensor_add(tmp_a[:], ps_a[:], sb_bin[:, 0:C])
                    nc.vector.tensor_mul(xg[:, m, :], tmp_a[:], sig[:])
                else:
                    nc.scalar.activation(sig[:], ps_g[:],
                                         mybir.ActivationFunctionType.Sigmoid)
                    nc.vector.tensor_mul(xg[:, m, :], ps_a[:], sig[:])

            def transpose_xg(m):
                # xg(m) -> xgT chunk slice via PE transpose + scalar copy
                c = m // 4
                if m % 4 == 0:
                    xgT_chunks[c] = xgTp.tile([P, 4, 512], BF, tag="xgT",
                                              name=f"xgT{c}")
                pxgT = ps_tr.tile([P, 4, P], BF, tag="tr")
                for q in range(4):
                    nc.tensor.transpose(pxgT[:, q, :], xg[:, m, ts(q, P)],
                                        sb_id16[:])
                nc.scalar.copy(xgT_chunks[c][:, :, ts(m % 4, P)], pxgT[:])

            e2_chunks = {}

            def weights_mm(n):
                # dynamic-weight logits for tokens [512n, 512n+512) in the
                # C-major [hk, t] domain + exp on ACT (logits are bounded, no
                # max-subtract); the transpose/normalize runs next iteration.
                xgTc = xgT_chunks[n]
                pw2 = ps_wl.tile([HK, 512], F32, tag="w1")
                for q in range(4):
                    nc.tensor.matmul(pw2[:], sb_wwtT[:, q, :], xgTc[:, q, :],
                                     start=(q == 0), stop=(q == 3))
                e2 = work.tile([HK, 512], BF, tag="e2", name=f"e2_{n}")
                if with_bias_wt:
                    nc.scalar.activation(e2[:], pw2[:],
                                         mybir.ActivationFunctionType.Exp,
                                         bias=sb_bwt[:])
                else:
                    nc.scalar.activation(e2[:], pw2[:],
                                         mybir.ActivationFunctionType.Exp)
                e2_chunks[n] = e2

            def weights_finish(n):
                # PE transpose of exp(logits) to token-major, then softmax
                # normalization on DVE: sum over K, 1/s, broadcast multiply.
                e2 = e2_chunks.pop(n)
                ptr = ps_tr.tile([P, 4, HK], BF, tag="tr", name="ptr")
                for j in range(4):
                    nc.tensor.transpose(ptr[:, j, :], e2[:, ts(j, P)],
                                        sb_id16[0:HK, 0:HK])
                pv = ptr[:].rearrange("p m (h k) -> p m h k", k=K)
                s8 = work.tile([P, 4, H], F32, tag="s8")
                nc.vector.tensor_reduce(s8[:], pv, mybir.AxisListType.X,
                                        mybir.AluOpType.add)
                r8 = work.tile([P, 4, H], F32, tag="r8")
                nc.vector.reciprocal_approx_fast(r8[:], s8[:])
                w_dst = wsm3[:, :, ts(n, 4), :].transpose([0, 2, 3, 1])
                nc.vector.tensor_tensor(
                    w_dst, pv, r8[:, :, :, None].to_broadcast((P, 4, H, K)),
                    mybir.AluOpType.mult)

            def build_dmas(mlo, mhi):
                # shifted copies of wsm3 feeding the band scatter, for time
                # tiles [mlo, mhi); main copies issue from sync (HWDGE), halo
                # wrap copies from vector, keeping gpsimd scatter-only
                for i in range(K):
                    d = i - 3
                    kk = 6 - i
                    if d == 0:
                        nc.sync.dma_start(data_tmp[:, i, mlo:mhi, :],
                                          wsm3[:, kk, mlo:mhi, :])
                    elif d < 0:
                        nc.sync.dma_start(data_tmp[-d:P, i, mlo:mhi, :],
                                          wsm3[0:P + d, kk, mlo:mhi, :])
                        lo = max(mlo, 1)
                        if lo < mhi:
                            nc.sync.dma_start(data_tmp[0:-d, i, lo:mhi, :],
                                              wsm3[P + d:P, kk, lo - 1:mhi - 1, :])
                    else:
                        nc.sync.dma_start(data_tmp[0:P - d, i, mlo:mhi, :],
                                          wsm3[d:P, kk, mlo:mhi, :])
                        hi = min(mhi, NT - 1)
                        if mlo < hi:
                            nc.sync.dma_start(data_tmp[P - d:P, i, mlo:hi, :],
                                              wsm3[0:d, kk, mlo + 1:hi + 1, :])

            def build_permute(mlo, mhi):
                # permute [p, i, m, h] -> [p, m, (i, h)]
                da4 = data_all[:, mlo:mhi, :].rearrange("p m (i h) -> p m i h",
                                                        h=H)
                nc.vector.tensor_copy(
                    da4, data_tmp[:, :, mlo:mhi, :].transpose([0, 2, 1, 3]))

            dt_tiles = {}
            el_tiles = {}

            def scatter_tile(m):
                dt = dtp.tile([P, DT_W], BF, tag="dt", name=f"dt{m}")
                nc.gpsimd.local_scatter(dt[:], data_all[:, m, :], sb_idxs[:],
                                        channels=P, num_elems=DT_W, num_idxs=HK)
                dt_tiles[m] = dt

            def conv_tile(m):
                dt = dt_tiles.pop(m)
                # [128, 4, 256] f32 = two PSUM banks; each 134-wide plane pair
                # stays inside a single bank
                pc = ps_c.tile([P, 4, 256], F32, tag="pc", name=f"pc{m}")
                pcv = pc[:, :, 0:CW]
                for ci in range(4):
                    for hp, pb in ((0, 0), (1, 64)):
                        hh = ci * 2 + hp
                        nc.tensor.matmul(
                            pcv[pb:pb + 64, ci, :], xg[:, m, ts(hh, 64)],
                            dt[:, MAIN_W * hh:MAIN_W * hh + CW],
                            start=True, stop=True, skip_group_check=True)
                t0 = m * P
                if m >= 1:
                    # right edge of tile m-1 first: it unblocks mm_out(m-1)
                    dr = conv[:, :, t0 - PAD_L:t0]
                    nc.vector.tensor_add(dr, dr, pcv[:, :, 0:PAD_L])
                # body of tile m (must precede its left-edge add)
                if with_conv_bias:
                    for ci in range(4):
                        nc.vector.tensor_scalar_add(
                            conv[:, ci, t0:t0 + P], pcv[:, ci, PAD_L:PAD_L + P],
                            sb_cb4[:, ci:ci + 1])
                else:
                    nc.vector.tensor_copy(conv[:, :, t0:t0 + P],
                                          pcv[:, :, PAD_L:PAD_L + P])
                if m - 1 in el_tiles:
                    # left edge of tile m: tile m-1 rows feeding t0..t0+2
                    dl = conv[:, :, t0:t0 + PAD_L]
                    nc.vector.tensor_add(dl, dl, el_tiles.pop(m - 1)[:])
                if m + 1 < NT:
                    # stage the outgoing right-edge so pc needs one generation
                    el = work.tile([P, 4, PAD_L], F32, tag="el", name=f"el{m}")
                    nc.vector.tensor_copy(el[:], pcv[:, :, CW - PAD_L:CW])
                    el_tiles[m] = el

            def mm_out(m):
                po = ps_o.tile([P, C], F32, tag="po")
                for q in range(4):
                    nc.tensor.matmul(po[:], conv[:, q, ts(m, P)],
                                     sb_woutT[:, q, :],
                                     start=(q == 0), stop=(q == 3))
                out_t = outp.tile([P, C], F32, tag="out_t")
                if with_bias_out:
                    nc.vector.tensor_add(out_t[:], po[:], sb_bout[:])
                else:
                    nc.vector.tensor_copy(out_t[:], po[:])
                nc.scalar.dma_start(y_d[ts(m, P), :], out_t[:])

            # ---- software-pipelined main loop ----
            # iter m: softmax-finish + build batch (every 4th) | conv(m-8) |
            #         tr(m-1) | scatter(m-7) | mm1(m) | logits chunk (every
            #         4th, before mm_out so exp hides under the iter tail) |
            #         mm_out(m-9)
            prefetch_chunk(0)
            SC_LAG, CONV_LAG, OUT_LAG = 9, 10, 11
            for m in range(NT + OUT_LAG + 2):
                if m % 8 == 1 and m // 8 + 1 < t_len // 1024:
                    prefetch_chunk(m // 8 + 1)
                if m % 4 == 1 and 5 <= m and (m - 5) // 4 < NC:
                    weights_finish((m - 5) // 4)
                bb = None
                if m % 4 == 2 and m >= 6:
                    # batch n covers tiles [4n-1, 4n+3), built one iter after
                    # its softmax; the last batch covers just the final tile
                    n = (m - 6) // 4
                    bb = (max(4 * n - 1, 0), min(4 * n + 3, NT))
                    if bb[0] < bb[1]:
                        build_dmas(*bb)
                    else:
                        bb = None
                if CONV_LAG <= m < NT + CONV_LAG:
                    conv_tile(m - CONV_LAG)
                if 1 <= m <= NT:
                    transpose_xg(m - 1)
                if SC_LAG <= m < NT + SC_LAG:
                    scatter_tile(m - SC_LAG)
                if m < NT:
                    mm1_glu(m)
                if m % 4 == 0 and 4 <= m <= NT:
                    weights_mm(m // 4 - 1)
                if OUT_LAG <= m <= NT - 1 + OUT_LAG:
                    mm_out(m - OUT_LAG)
                if bb is not None:
                    build_permute(*bb)

    nc.compile()
    return nc


def host_inputs(x_b, w_in, b_in, w_wt, b_wt, w_out, b_out, conv_bias,
                with_bias_in, with_bias_wt, with_bias_out, with_conv_bias):
    """Per-core input map from a batch slice + shared weights."""
    def t_pack(w, width, dt_=None):
        # w: [width, C] -> [128, 4, width] with [p, q, f] = w[f, 128q+p]
        a = np.ascontiguousarray(
            w.T.reshape(4, P, width).transpose(1, 0, 2)).astype(dt_ or BF16)
        return a

    m = {
        "xT": np.ascontiguousarray(np.asarray(x_b, np.float32).T).astype(BF16),
        "w_inT": t_pack(w_in, C2),
        "w_wtT": t_pack(w_wt, HK),
        "w_outT": t_pack(w_out, C),
        "idxs": host_scatter_idxs(),
        "ident16": np.eye(P).astype(BF16),
    }
    if with_bias_in:
        m["b_in"] = np.asarray(b_in, np.float32)
    if with_bias_wt:
        m["b_wt"] = np.asarray(b_wt, np.float32)
    if with_bias_out:
        m["b_out"] = np.asarray(b_out, np.float32)
    if with_conv_bias:
        m["cb4"] = np.ascontiguousarray(
            np.asarray(conv_bias, np.float32).reshape(4, P).T)
    return m


_NC_CACHE = {}


def _get_nc(key):
    if key not in _NC_CACHE:
        _NC_CACHE[key] = build_nc(T, *key)
    return _NC_CACHE[key]


def kernel(x, w_in, b_in, w_wt, b_wt, w_out, b_out, conv_bias, _trace=False):
    x = np.asarray(x)
    flags = (bool(np.any(b_in)), bool(np.any(b_wt)), bool(np.any(b_out)),
             bool(np.any(conv_bias)))
    nc = _get_nc(flags)
    in_maps = [
        host_inputs(x[:, b, :], np.asarray(w_in), b_in, np.asarray(w_wt), b_wt,
                    np.asarray(w_out), b_out, conv_bias, *flags)
        for b in range(B)
    ]
    res = run_bass_kernel_spmd(nc, in_maps, core_ids=list(range(B)),
                               trace=_trace)
    y = np.stack([np.asarray(res.results[b]["y"]) for b in range(B)], axis=1)
    if _trace:
        return y.astype(np.float32), res
    return y.astype(np.float32)
